# revision 4
# baseline (speedup 1.0000x reference)
"""ExtraMSAEmbedding Trainium2 kernel.

out[s, r, :] = one_hot(msa[s, r], 23) @ W[:, :23].T
             + has_del[s, r] * W[:, 23] + del_val[s, r] * W[:, 24] + b

Strategy (8 NeuronCores, data-parallel over the 2048 extra sequences — 256
seqs = 98304 tokens per core):

- tokens are processed in blocks of 512, 4 blocks ("groups") per iteration
- a K=1 matmul on the TensorEngine broadcasts each block's msa values (as
  f32) across 32 PSUM partitions (lhsT is a 0/1 mask row so rows >= 23 get
  0.0)
- one DVE tensor_scalar(is_equal) against a per-partition class-index
  column turns that into the 23-row one-hot, transposed ([class, token]
  layout).  Partition row 25 compares 0==0 -> constant 1.0 (bias row);
  rows 23/24 are overwritten by direct DMA of has_del / del_val.
- the embedding is then a single K=26 matmul per block with stationary
  weights [W.T ; b] producing out.T tiles [64 channels, 512 tokens].
  The 4 block-matmuls + 4 broadcast-matmuls of an iteration are placed on
  disjoint 32x32 PE subarrays via tile_position so they can overlap.
- ScalarE (ACT) copies PSUM->SBUF, DMA stores channel-major [64, T] per
  core (perfectly contiguous 2 KB runs); the host transposes when
  unsharding.
"""

import numpy as np

N_SEQ, N_RES = 2048, 384
C_OUT = 64
N_CORES = 8
SEQ_PER_CORE = N_SEQ // N_CORES  # 256
T_PER_CORE = SEQ_PER_CORE * N_RES  # 98304
BLK = 512  # tokens per block (one PSUM bank of f32)
N_BLOCKS = T_PER_CORE // BLK  # 192
GROUPS = 4  # blocks per iteration
# group g's msa staging row lives at partition 32*PI[g] (chosen so the
# broadcast matmuls land on PE subarrays disjoint from the main matmuls)
PI = [(g + 2) % 4 for g in range(GROUPS)]

_CACHE: dict = {}
_LAST_RESULT = None


def build_program(n_blocks: int = N_BLOCKS):
    """Build + compile the Bass/Tile program (same program for all cores)."""
    import concourse.bass as bass  # noqa: F401
    import concourse.mybir as mybir
    import concourse.tile as tile
    from concourse import bacc

    f32 = mybir.dt.float32
    assert n_blocks % GROUPS == 0
    n_iters = n_blocks // GROUPS

    nc = bacc.Bacc("TRN2", target_bir_lowering=False, debug=False)

    msa_d = nc.dram_tensor("msa", [n_blocks, BLK], f32, kind="ExternalInput").ap()
    has_d = nc.dram_tensor("hasdel", [n_blocks, BLK], f32, kind="ExternalInput").ap()
    del_d = nc.dram_tensor("delval", [n_blocks, BLK], f32, kind="ExternalInput").ap()
    w26_d = nc.dram_tensor("w26", [128, C_OUT], f32, kind="ExternalInput").ap()
    mask_d = nc.dram_tensor("mask", [128, 32], f32, kind="ExternalInput").ap()
    ccol_d = nc.dram_tensor("ccol", [128, 1], f32, kind="ExternalInput").ap()
    out_d = nc.dram_tensor(
        "out", [C_OUT, n_blocks * BLK], f32, kind="ExternalOutput"
    ).ap()
    # [block, channel, token] view of the channel-major output
    out_v = out_d.rearrange("o (nb t) -> nb o t", t=BLK)

    with tile.TileContext(nc) as tc:
        with (
            tc.tile_pool(name="consts", bufs=1) as cpool,
            tc.tile_pool(name="staging", bufs=3) as spool,
            tc.tile_pool(name="feat", bufs=3) as fpool,
            tc.tile_pool(name="osb", bufs=3) as opool,
            tc.tile_pool(name="pbc", bufs=2, space=bass.MemorySpace.PSUM) as pbpool,
            tc.tile_pool(name="pout", bufs=2, space=bass.MemorySpace.PSUM) as popool,
        ):
            w26 = cpool.tile([128, C_OUT], f32)
            nc.sync.dma_start(w26[:], w26_d)
            mask = cpool.tile([128, 32], f32)
            nc.sync.dma_start(mask[:], mask_d)
            ccol = cpool.tile([128, 1], f32)
            nc.sync.dma_start(ccol[:], ccol_d)

            for i in range(n_iters):
                b0 = GROUPS * i
                # msa staging rows at partitions {0,32,64,96}; host supplies
                # blocks permuted so partition 32*p holds block b0+(p+2)%4,
                # i.e. group g sits at partition 32*PI[g].
                staging = spool.tile([128, BLK], f32)
                nc.sync.dma_start(staging[0:128:32, :], msa_d[b0 : b0 + 4, :])

                # broadcast matmuls: psum_b[32g+j, t] = mask[j] * msa_g[t]
                pb = pbpool.tile([128, BLK], f32)
                for g in range(GROUPS):
                    pg = 32 * PI[g]
                    nc.tensor.matmul(
                        pb[32 * g : 32 * g + 32, :],
                        mask[pg : pg + 1, :],
                        staging[pg : pg + 1, :],
                        tile_position=(pg, 32 * g),
                    )

                # one-hot (+ ones row 25) via is_equal against class column
                feat = fpool.tile([128, BLK], f32)
                nc.vector.tensor_scalar(
                    feat[:], pb[:], ccol[:], None, mybir.AluOpType.is_equal
                )
                # deletion features into rows 23/24 of each 32-row group
                nc.sync.dma_start(feat[23:128:32, :], has_d[b0 : b0 + 4, :])
                nc.sync.dma_start(feat[24:128:32, :], del_d[b0 : b0 + 4, :])

                # main matmuls: out.T[64, 512] = W26.T @ feat_g  (K=26)
                po = [
                    popool.tile([128, BLK], f32, tag="po0", name="po0"),
                    popool.tile([128, BLK], f32, tag="po1", name="po1"),
                ]
                for g in range(GROUPS):
                    bank, half = g % 2, 64 * (g // 2)
                    nc.tensor.matmul(
                        po[bank][half : half + 64, :],
                        w26[32 * g : 32 * g + 26, :],
                        feat[32 * g : 32 * g + 26, :],
                        tile_position=(32 * g, half),
                    )

                # PSUM -> SBUF on ScalarE, then store channel-major
                for bank in range(2):
                    osb = opool.tile([128, BLK], f32, tag=f"osb{bank}")
                    nc.scalar.copy(osb[:], po[bank][:])
                    nc.sync.dma_start(out_v[b0 + bank : b0 + bank + 3 : 2], osb[:])

    nc.compile()
    return nc


def _host_constants(W: np.ndarray, b: np.ndarray):
    f32 = np.float32
    w26 = np.zeros((32, C_OUT), f32)
    w26[0:25] = W.T.astype(f32)  # rows 0-22 one-hot classes, 23 has, 24 del
    w26[25] = b.astype(f32)  # ones-row -> bias
    w26 = np.tile(w26, (4, 1))  # replicate for the 4 K-strips

    mask = np.zeros((128, 32), f32)
    mask[:, 0:23] = 1.0  # broadcast only class rows; rows 23-31 get 0

    ccol = np.full((128, 1), -7.0, f32)
    for p in range(128):
        j = p % 32
        if j < 23:
            ccol[p] = j  # one-hot compare value
        elif j == 25:
            ccol[p] = 0.0  # matches the broadcast 0 -> constant 1.0
    return w26, mask, ccol


def _permute_blocks(x_blocks: np.ndarray) -> np.ndarray:
    """Reorder blocks within each group of 4 as [2,3,0,1] (staging order)."""
    nb = x_blocks.shape[0]
    return (
        x_blocks.reshape(nb // 4, 4, BLK)[:, [2, 3, 0, 1], :]
        .reshape(nb, BLK)
        .copy()
    )


def kernel(extra_msa, extra_has_deletion, extra_deletion_value, W, b):
    from concourse.bass_utils import run_bass_kernel_spmd

    f32 = np.float32
    msa = np.asarray(extra_msa).astype(f32)  # int -> f32 (exact for 0..22)
    has_ = np.asarray(extra_has_deletion, dtype=f32)
    del_ = np.asarray(extra_deletion_value, dtype=f32)
    W = np.asarray(W, dtype=f32)
    b = np.asarray(b, dtype=f32)

    if "nc" not in _CACHE:
        _CACHE["nc"] = build_program(N_BLOCKS)
    nc = _CACHE["nc"]

    w26, mask, ccol = _host_constants(W, b)

    in_maps = []
    for c in range(N_CORES):
        s0, s1 = c * SEQ_PER_CORE, (c + 1) * SEQ_PER_CORE
        in_maps.append(
            {
                "msa": _permute_blocks(msa[s0:s1].reshape(N_BLOCKS, BLK)),
                "hasdel": np.ascontiguousarray(has_[s0:s1].reshape(N_BLOCKS, BLK)),
                "delval": np.ascontiguousarray(del_[s0:s1].reshape(N_BLOCKS, BLK)),
                "w26": w26,
                "mask": mask,
                "ccol": ccol,
            }
        )

    res = run_bass_kernel_spmd(nc, in_maps, list(range(N_CORES)))
    global _LAST_RESULT
    _LAST_RESULT = res
    # unshard: [64, T] channel-major per core -> [256, 384, 64] each
    parts = [
        r["out"].T.reshape(SEQ_PER_CORE, N_RES, C_OUT) for r in res.results
    ]
    return np.ascontiguousarray(np.concatenate(parts, axis=0))


# revision 11
# speedup vs baseline: 4.9916x; 4.9916x over previous
"""ExtraMSAEmbedding Trainium2 kernel.

out[s, r, :] = one_hot(msa[s, r], 23) @ W[:, :23].T
             + has_del[s, r] * W[:, 23] + del_val[s, r] * W[:, 24] + b

Strategy (8 NeuronCores, data-parallel over the 2048 extra sequences — 256
seqs = 98304 tokens per core):

- tokens are processed in blocks of 512 (one PSUM bank), 4 blocks
  ("groups" g=0..3) per iteration, SUPER=8 iterations per DMA batch.
- a K=1 matmul on the TensorEngine broadcasts each block's msa values (as
  f32) across 32 PSUM partitions (lhsT is a 0/1 mask row so rows >= 23 get
  0.0)
- one DVE tensor_scalar(is_equal) per iteration against a per-partition
  class-index column turns that into the transposed one-hot
  ([class, token] layout).  Partition row 25 compares 0==0 -> constant
  1.0 (bias row); rows 23/24 are overwritten by DMA of has_del/del_val
  (batched once per super-block).
- the embedding is a single K=26 matmul per block with stationary weights
  [W.T ; b] producing out.T tiles [64 channels, 512 tokens]. The 4 block
  matmuls + 4 broadcast matmuls of an iteration sit on disjoint 32x32 PE
  subarrays via tile_position, so they overlap on the array.
- ScalarE (ACT) copies PSUM->SBUF into big [128, SUPER*512] staging
  tiles; outputs leave as raw [iter, bank, 128, 512] dumps via SWDGE
  (gpsimd) DMA — descriptors spread over all 16 SDMA engines.  The host
  does the final (cheap) layout transpose while unsharding.
"""

import numpy as np

N_SEQ, N_RES = 2048, 384
C_OUT = 64
N_CORES = 8
SEQ_PER_CORE = N_SEQ // N_CORES  # 256
T_PER_CORE = SEQ_PER_CORE * N_RES  # 98304
BLK = 512  # tokens per block (one PSUM bank of f32)
N_BLOCKS = T_PER_CORE // BLK  # 192
GROUPS = 4  # blocks per iteration
SUPER = 8  # iterations per DMA batch
# group g's msa staging row lives at partition 32*PI[g] (chosen so the
# broadcast matmuls land on PE subarrays disjoint from the main matmuls)
PI = [(g + 2) % 4 for g in range(GROUPS)]

_CACHE: dict = {}
_LAST_RESULT = None


def build_program(n_blocks: int = N_BLOCKS):
    """Build + compile the Bass/Tile program (same program for all cores)."""
    import concourse.bass as bass  # noqa: F401
    import concourse.mybir as mybir
    import concourse.tile as tile
    from concourse import bacc

    f32 = mybir.dt.float32
    bf16 = mybir.dt.bfloat16
    assert n_blocks % (GROUPS * SUPER) == 0
    n_super = n_blocks // (GROUPS * SUPER)
    FREE = SUPER * BLK  # free-dim of the big per-super tiles

    nc = bacc.Bacc("TRN2", target_bir_lowering=False, debug=False)

    # inputs laid out per super-block by the host (see kernel() below)
    # msa in bf16: exact for integers 0..22, makes the broadcast matmul a
    # single-pass bf16 matmul instead of a two-pass fp32 one
    msa_d = nc.dram_tensor(
        "msa", [n_super, GROUPS, SUPER, BLK], bf16, kind="ExternalInput"
    ).ap()
    has_d = nc.dram_tensor(
        "hasdel", [n_super, GROUPS, SUPER, BLK], f32, kind="ExternalInput"
    ).ap()
    del_d = nc.dram_tensor(
        "delval", [n_super, GROUPS, SUPER, BLK], f32, kind="ExternalInput"
    ).ap()
    w26_d = nc.dram_tensor("w26", [128, C_OUT], f32, kind="ExternalInput").ap()
    mask_d = nc.dram_tensor("mask", [128, 32], bf16, kind="ExternalInput").ap()
    ccol_d = nc.dram_tensor("ccol", [128, 1], f32, kind="ExternalInput").ap()
    # raw output dump: [iter, 128 partitions, 1024] (host fixes layout)
    out_d = nc.dram_tensor(
        "out", [n_blocks // GROUPS, 128, 2 * BLK], f32, kind="ExternalOutput"
    ).ap()
    # [partition, iter, free] view for per-super stores
    out_v = out_d.rearrange("i p f -> p i f")

    with tile.TileContext(nc) as tc:
        with (
            tc.tile_pool(name="consts", bufs=1) as cpool,
            tc.tile_pool(name="staging", bufs=2) as spool,
            tc.tile_pool(name="feat", bufs=2) as fpool,
            tc.tile_pool(name="osb", bufs=2) as opool,
            tc.tile_pool(name="pbc", bufs=2, space=bass.MemorySpace.PSUM) as pbpool,
            tc.tile_pool(name="pout", bufs=2, space=bass.MemorySpace.PSUM) as popool,
        ):
            w26 = cpool.tile([128, C_OUT], f32)
            nc.sync.dma_start(w26[:], w26_d)
            mask = cpool.tile([128, 32], bf16)
            nc.sync.dma_start(mask[:], mask_d)
            ccol = cpool.tile([128, 1], f32)
            nc.sync.dma_start(ccol[:], ccol_d)

            for s in range(n_super):
                # big input staging: partition 32p holds msa of group (p+2)%4
                # for the 8 iterations of this super-block
                staging = spool.tile([128, FREE], bf16)
                nc.sync.dma_start(staging[0:128:32, :], msa_d[s])

                feat = fpool.tile([128, FREE], f32)
                for jj in range(SUPER // 2):
                    # two iterations share one 2-bank PSUM tile so the DVE
                    # eq op runs at FD=1024
                    pb = pbpool.tile([128, 2 * BLK], f32, name="pb")
                    for j2 in range(2):
                        j = 2 * jj + j2
                        cs = slice(j * BLK, (j + 1) * BLK)
                        # broadcast matmuls: pb[32g+k, t] = mask[k]*msa_g[t]
                        for g in range(GROUPS):
                            pg = 32 * PI[g]
                            nc.tensor.matmul(
                                pb[32 * g : 32 * g + 32, j2 * BLK : (j2 + 1) * BLK],
                                mask[pg : pg + 1, :],
                                staging[pg : pg + 1, cs],
                                tile_position=(pg, 32 * g),
                            )
                    # one-hot (+ ones row 25) via is_equal vs class column
                    nc.vector.tensor_scalar(
                        feat[:, 2 * jj * BLK : (2 * jj + 2) * BLK],
                        pb[:],
                        ccol[:],
                        None,
                        mybir.AluOpType.is_equal,
                    )

                # deletion features into rows 23/24 of each 32-row group
                # (after the eq ops in program order; Tile serializes the
                # overlapping writes correctly)
                nc.sync.dma_start(feat[23:128:32, :], has_d[s])
                nc.sync.dma_start(feat[24:128:32, :], del_d[s])

                # osb layout per partition: [iter j | bank | 512 tokens]
                osb = opool.tile([128, SUPER * 2 * BLK], f32, name="osb")
                for j in range(SUPER):
                    cs = slice(j * BLK, (j + 1) * BLK)
                    # main matmuls: out.T[64, 512] = W26.T @ feat_g (K=26)
                    po = popool.tile([128, 2 * BLK], f32, name="po")
                    for g in range(GROUPS):
                        bank, half = g % 2, 64 * (g // 2)
                        nc.tensor.matmul(
                            po[half : half + 64, bank * BLK : (bank + 1) * BLK],
                            w26[32 * g : 32 * g + 26, :],
                            feat[32 * g : 32 * g + 26, cs],
                            tile_position=(32 * g, half),
                        )
                    nc.scalar.copy(
                        osb[:, j * 2 * BLK : (j + 1) * 2 * BLK], po[:]
                    )

                # raw store via SWDGE: descriptors spread over 16 SDMA engines
                nc.gpsimd.dma_start(
                    out_v[:, s * SUPER : (s + 1) * SUPER, :], osb[:]
                )

    nc.compile()
    return nc


def _host_constants(W: np.ndarray, b: np.ndarray):
    import ml_dtypes

    f32 = np.float32
    w26 = np.zeros((32, C_OUT), f32)
    w26[0:25] = W.T.astype(f32)  # rows 0-22 one-hot classes, 23 has, 24 del
    w26[25] = b.astype(f32)  # ones-row -> bias
    w26 = np.tile(w26, (4, 1))  # replicate for the 4 K-strips

    mask = np.zeros((128, 32), ml_dtypes.bfloat16)
    mask[:, 0:23] = 1.0  # broadcast only class rows; rows 23-31 get 0

    ccol = np.full((128, 1), -7.0, f32)
    for p in range(128):
        j = p % 32
        if j < 23:
            ccol[p] = j  # one-hot compare value
        elif j == 25:
            ccol[p] = 0.0  # matches the broadcast 0 -> constant 1.0
    return w26, mask, ccol


def _stage_blocks(x_blocks: np.ndarray, perm: bool) -> np.ndarray:
    """[n_blocks, BLK] -> [n_super, GROUPS, SUPER, BLK] staging layout.

    Element [s, p, j] = block 4*(SUPER*s + j) + g  with g = (p+2)%4 when
    perm (msa staging partition order), else g = p (feat row order).
    """
    nb = x_blocks.shape[0]
    x = x_blocks.reshape(nb // (GROUPS * SUPER), SUPER, GROUPS, BLK)
    x = x.transpose(0, 2, 1, 3)  # [s, g, j, t]
    if perm:
        x = x[:, [2, 3, 0, 1], :, :]  # partition p holds group (p+2)%4
    return np.ascontiguousarray(x)


def kernel(extra_msa, extra_has_deletion, extra_deletion_value, W, b):
    from concourse.bass_utils import run_bass_kernel_spmd

    f32 = np.float32
    msa = np.asarray(extra_msa).astype(f32)  # int -> f32 (exact for 0..22)
    has_ = np.asarray(extra_has_deletion, dtype=f32)
    del_ = np.asarray(extra_deletion_value, dtype=f32)
    W = np.asarray(W, dtype=f32)
    b = np.asarray(b, dtype=f32)

    if "nc" not in _CACHE:
        _CACHE["nc"] = build_program(N_BLOCKS)
    nc = _CACHE["nc"]

    w26, mask, ccol = _host_constants(W, b)

    import ml_dtypes

    in_maps = []
    for c in range(N_CORES):
        s0, s1 = c * SEQ_PER_CORE, (c + 1) * SEQ_PER_CORE
        in_maps.append(
            {
                "msa": _stage_blocks(msa[s0:s1].reshape(N_BLOCKS, BLK), True).astype(
                    ml_dtypes.bfloat16
                ),
                "hasdel": _stage_blocks(has_[s0:s1].reshape(N_BLOCKS, BLK), False),
                "delval": _stage_blocks(del_[s0:s1].reshape(N_BLOCKS, BLK), False),
                "w26": w26,
                "mask": mask,
                "ccol": ccol,
            }
        )

    res = run_bass_kernel_spmd(nc, in_maps, list(range(N_CORES)))
    global _LAST_RESULT
    _LAST_RESULT = res

    # unshard: raw [iter, 128, 1024] -> token-major [256, 384, 64]
    parts = []
    for r in res.results:
        raw = r["out"].reshape(N_BLOCKS // GROUPS, 2, C_OUT, 2, BLK)
        # axes (i, half, ch, bank, t): block = 4i + 2*half + bank
        tok = raw.transpose(0, 1, 3, 4, 2).reshape(T_PER_CORE, C_OUT)
        parts.append(tok.reshape(SEQ_PER_CORE, N_RES, C_OUT))
    return np.ascontiguousarray(np.concatenate(parts, axis=0))
